# revision 19
# baseline (speedup 1.0000x reference)
"""BiLSTM-CRF loss kernel for Trainium2, 8-core data parallel.

Feature-major (transposed) layout throughout: gates live on partitions,
batch on the free dim, so elementwise ops use all 128 lanes and h_t is
produced already transposed for the next step's matmul (no per-step PE
transposes or staging copies).

Per-core (batch shard of 32, both LSTM directions as independent chains):
  P2 (with the input projection fused in): per 4-step window and
      direction, the x-projection W_ih x^T (+ rank-1 bias) is accumulated
      straight into a PSUM window tile; each LSTM step adds the recurrent
      part with 16 [128x128]x[128x32] matmuls, then sigmoid/tanh on
      [128, 6*32]/[128, 2*32] tiles and a 4-op DVE cell update writing
      h^T straight into the hts buffer.
  P3: emission matmuls + gold-path dot (ps * onehot reduce) + exp(em)
      fused from PSUM into a bf16 emission buffer; blocks emitted
      outside-in so the CRF chains can chase them.
  P4: CRF partition function in scaled linear space with an absorbing
      77th tag, split into a forward-alpha chain (t=0..64) and a
      backward-beta chain (t=127..65) joined in the middle; bf16
      alpha/transition factors; final log + reductions.
Host combines the 8 partial sums into the scalar loss.
"""

import numpy as np
import ml_dtypes

import concourse.bass as bass
import concourse.mybir as mybir
from concourse.tile import TileContext
from concourse import library_config
from concourse.vector_clock import ScopedClock

N_CORES = 8
B, S, E, HD, T, V = 256, 128, 512, 256, 76, 30000
BC = B // N_CORES          # 32 batch per core
G4 = 4 * HD                # 1024 gates
TA = T + 1                 # 77 tags with absorber
NTOK = S * BC              # 4096 tokens per core (shared by both dirs)
NC = 8                     # gate chunks of 128
NW = S // 4                # 4-step windows

dt = mybir.dt
F32, BF16, I16 = dt.float32, dt.bfloat16, dt.int16
AF = mybir.ActivationFunctionType
ALU = mybir.AluOpType

# ---------------------------------------------------------------- tile patch
# This walrus build rejects >1 sem wait on CTRL-class (Drain/NoOp)
# instructions; split the Tile tail-drain waits across preceding NOPs.
_MAX_WAITS = 1


_WAIT_LIMITS = {}


def _split_excess_waits(nc):
    """Non-DMA instructions accept only one sem wait on this walrus build;
    move excess waits onto NOPs spliced in front (same engine, same order)."""
    for f in nc.m.functions:
        stack = list(f.blocks)
        while stack:
            bb = stack.pop()
            for sub in getattr(bb, "blocks", []) or []:
                stack.append(sub)
            insts = getattr(bb, "instructions", None)
            if not insts:
                continue
            newlist = []
            changed = False
            for inst in insts:
                si = inst.sync_info
                lim = _WAIT_LIMITS.get(type(inst).__name__, 1)
                if si is not None and si.on_wait and len(si.on_wait) > lim:
                    waits = list(si.on_wait)
                    si.on_wait = waits[-lim:]
                    for w in waits[:-lim]:
                        nop = mybir.InstNoOp(
                            name=f"I-wsplit{nc.next_id()}", ins=[], outs=[],
                            engine=inst.engine,
                            sync_info=mybir.SyncInfo(on_wait=[w], on_update=[]),
                        )
                        newlist.append(nop)
                    changed = True
                newlist.append(inst)
            if changed:
                insts[:] = newlist


def _patched_drain_and_barrier(self, tick_clock, wait_clock):
    nc = self.nc
    _split_excess_waits(nc)
    nops = [nc.sync.nop(nofuse=True, hint=f"waitsplit{i}") for i in range(16)]
    drain_inst = nc.sync.drain()
    wait_clock.add_sem_waits(
        drain_inst.ins, ScopedClock({None: tick_clock.global_clock})
    )
    si = drain_inst.ins.sync_info
    if si is not None and si.on_wait and len(si.on_wait) > _MAX_WAITS:
        waits = list(si.on_wait)
        chunks = [waits[i:i + _MAX_WAITS] for i in range(0, len(waits), _MAX_WAITS)]
        si.on_wait = chunks[-1]
        assert len(chunks) - 1 <= len(nops), "too many wait chunks"
        for i, ch in enumerate(chunks[:-1]):
            ni = nops[i].ins
            if ni.sync_info is None:
                ni.sync_info = mybir.SyncInfo(on_wait=ch, on_update=[])
            else:
                ni.sync_info.on_wait = list(ni.sync_info.on_wait) + ch
    nc.all_engine_barrier()
    assert self.sems is not None
    popped = nc._tile_sem_poison_stack.pop()
    assert popped is self._sem_poison
    allsems = list(self.sems.allocated().values())
    for i in range(0, len(allsems), 8):
        nc.clear_and_free_semaphores(allsems[i:i + 8])
    nc.all_engine_barrier()


def apply_tile_patch():
    TileContext._drain_and_barrier = _patched_drain_and_barrier


# ---------------------------------------------------------------- builder
def build_nc():
    apply_tile_patch()
    nc = bass.Bass("TRN2", target_bir_lowering=False, debug=False,
                   num_devices=N_CORES)

    xt_d = nc.dram_tensor("xt", [128, 4, NTOK], BF16, kind="ExternalInput")
    wih = nc.dram_tensor("wih", [128, 2, 4, NC, 128], BF16,
                         kind="ExternalInput")
    whh = nc.dram_tensor("whh", [128, 2, 2, NC, 128], BF16,
                         kind="ExternalInput")
    brow = nc.dram_tensor("brow", [1, 2, NC, 128], BF16, kind="ExternalInput")
    h0t = nc.dram_tensor("h0t", [128, 2, 2, BC], BF16, kind="ExternalInput")
    c0t = nc.dram_tensor("c0t", [128, 2, 2, BC], F32, kind="ExternalInput")
    wout = nc.dram_tensor("wout", [128, 4, T], BF16, kind="ExternalInput")
    # tables: [trans(0:76) | start(76) | end(77) | bout(78) | negkappa(79)]
    tables = nc.dram_tensor("tables", [T, 80], F32, kind="ExternalInput")
    gcnt = nc.dram_tensor("gcnt", [T, 79], F32, kind="ExternalInput")
    ohm = nc.dram_tensor("ohm", [T, NTOK], BF16, kind="ExternalInput")
    vmask = nc.dram_tensor("vmask", [T, NTOK], BF16, kind="ExternalInput")
    padrow = nc.dram_tensor("padrow", [1, NTOK], BF16, kind="ExternalInput")
    mp_d = nc.dram_tensor("mp", [TA, TA], BF16, kind="ExternalInput")
    mpT_d = nc.dram_tensor("mpT", [TA, TA], BF16, kind="ExternalInput")
    eend_d = nc.dram_tensor("eend", [TA, 1], F32, kind="ExternalInput")
    out_d = nc.dram_tensor("out", [1, 2], F32, kind="ExternalOutput")

    with TileContext(nc) as tc:
        with (
            tc.tile_pool(name="const", bufs=1) as cpool,
            tc.tile_pool(name="work", bufs=3) as wpool,
            tc.tile_pool(name="state", bufs=3) as spool,
            tc.tile_pool(name="xq", bufs=1) as xqpool,
        ):
            zwps = tc.alloc_tile_pool(name="zwps", bufs=2, space="PSUM")
            # ---- constants / inputs into SBUF
            wih_sb = cpool.tile([128, 2, 4, NC, 128], BF16)
            nc.sync.dma_start(wih_sb[:], wih[:])
            whh_sb = cpool.tile([128, 2, 2, NC, 128], BF16)
            nc.sync.dma_start(whh_sb[:], whh[:])
            brow_sb = cpool.tile([1, 2, NC, 128], BF16)
            nc.sync.dma_start(brow_sb[:], brow[:])
            h0_sb = cpool.tile([128, 2, 2, BC], BF16)
            nc.sync.dma_start(h0_sb[:], h0t[:])
            c0_sb = cpool.tile([128, 2, 2, BC], F32)
            nc.sync.dma_start(c0_sb[:], c0t[:])
            wout_sb = cpool.tile([128, 4, T], BF16)
            nc.sync.dma_start(wout_sb[:], wout[:])
            tab_sb = cpool.tile([T, 80], F32)
            nc.sync.dma_start(tab_sb[:], tables[:])
            gcnt_sb = cpool.tile([T, 79], F32)
            nc.sync.dma_start(gcnt_sb[:], gcnt[:])
            mp_sb = cpool.tile([TA, TA], BF16)
            nc.sync.dma_start(mp_sb[:], mp_d[:])
            mpT_sb = cpool.tile([TA, TA], BF16)
            nc.sync.dma_start(mpT_sb[:], mpT_d[:])
            eend_sb = cpool.tile([TA, 1], F32)
            nc.sync.dma_start(eend_sb[:], eend_d[:])
            ohm_sb = cpool.tile([T, NTOK], BF16)
            nc.sync.dma_start(ohm_sb[:], ohm[:])
            vm_sb = cpool.tile([T, NTOK], BF16)
            nc.sync.dma_start(vm_sb[:], vmask[:])
            onesrow = cpool.tile([1, 128], BF16)
            nc.vector.memset(onesrow[:], 1.0)
            onesta = cpool.tile([TA, 1], BF16)
            nc.vector.memset(onesta[:], 1.0)
            em_sb = cpool.tile([TA, NTOK], BF16)
            hts = {0: cpool.tile([128, 2, NTOK], BF16, name="hft"),
                   1: cpool.tile([128, 2, NTOK], BF16, name="hbt")}

            # ---- P2 with the input projection fused via PSUM handoff
            zwd = {0: {}, 1: {}}
            xqd = {0: {}, 1: {}}
            CORD = [6, 7, 0, 1, 2, 3, 4, 5]   # tanh_g chunks complete first

            def emit_window_dma(w):
                for d in range(2):
                    blk = w if d == 0 else NW - 1 - w
                    xq = xqpool.tile([128, 4, 128], BF16, tag=f"xq{d}",
                                     bufs=4, name=f"xq{d}")
                    xqd[d][w] = xq
                    nc.sync.dma_start(
                        xq[:], xt_d.ap()[:, :, blk * 128:(blk + 1) * 128])

            def emit_window_chunks(w, cs):
                for d in range(2):
                    zw = zwd[d][w]
                    xq = xqd[d][w]
                    for c in cs:
                        for k in range(4):
                            nc.tensor.matmul(
                                zw[:, c, :], wih_sb[:, d, k, c, :],
                                xq[:, k, :], start=(k == 0), stop=False,
                                skip_group_check=True)
                        nc.tensor.matmul(
                            zw[:, c, :], brow_sb[:, d, c, :], onesrow[:],
                            start=False, stop=False, skip_group_check=True)

            emit_window_dma(0)
            emit_window_dma(1)
            for d in range(2):
                zwd[d][0] = zwps.tile([128, NC, 128], F32, tag=f"zw{d}", name=f"zw{d}")
            emit_window_chunks(0, range(NC))
            c_st = {0: c0_sb[:, 0], 1: c0_sb[:, 1]}
            for t in range(S):
                w, s = divmod(t, 4)
                if s == 0:
                    if w + 2 < NW:
                        emit_window_dma(w + 2)
                    if w + 1 < NW:
                        for d in range(2):
                            zwd[d][w + 1] = zwps.tile([128, NC, 128], F32,
                                                      tag=f"zw{d}",
                                                      name=f"zw{d}")
                if w + 1 < NW:
                    emit_window_chunks(w + 1, [2 * s, 2 * s + 1])
                sls = {}
                for d in range(2):
                    so = s if d == 0 else 3 - s
                    sls[d] = slice(so * BC, (so + 1) * BC)
                for d in range(2):
                    zw = zwd[d][w]
                    for c in CORD:
                        for k in range(2):
                            if t == 0:
                                hk = h0_sb[:, d, k, :]
                            elif d == 0:
                                pc = (t - 1) * BC
                                hk = hts[0][:, k, pc:pc + BC]
                            else:
                                pc = (S - t) * BC
                                hk = hts[1][:, k, pc:pc + BC]
                            nc.tensor.matmul(zw[:, c, sls[d]],
                                             whh_sb[:, d, k, c, :], hk,
                                             start=False, stop=(k == 1),
                                             skip_group_check=True)
                # cell layout: sig(i,f,o) 0:6 | sig(2g)->t1 6:8 | tg' 8:10
                #              | th 10:12; tanh_g comes from 2*sig(2g)-1
                # (g-gate weights/bias are doubled host-side).
                cells = {}
                for d in range(2):
                    zw = zwd[d][w]
                    cell = wpool.tile([128, 12, BC], BF16, tag=f"cell{d}",
                                      name=f"cell{d}", bufs=4)
                    cells[d] = cell
                    nc.scalar.activation(cell[:, 0:8, :], zw[:, 0:8, sls[d]],
                                         AF.Sigmoid)
                for d in range(2):
                    cell = cells[d]
                    c_old = c_st[d]
                    c_new = spool.tile([128, 2, BC], F32, tag=f"c{d}",
                                       name=f"c{d}", bufs=4)
                    nc.vector.tensor_scalar(cell[:, 8:10, :], cell[:, 6:8, :],
                                            2.0, -1.0, ALU.mult, ALU.add)
                    nc.vector.tensor_mul(cell[:, 6:8, :], cell[:, 0:2, :],
                                         cell[:, 8:10, :])
                    nc.vector.tensor_mul(c_new[:], cell[:, 2:4, :], c_old)
                    nc.vector.tensor_add(c_new[:], c_new[:], cell[:, 6:8, :])
                    c_st[d] = c_new[:]
                for d in range(2):
                    nc.scalar.activation(cells[d][:, 10:12, :], c_st[d],
                                         AF.Tanh)
                for d in range(2):
                    col = (t if d == 0 else S - 1 - t) * BC
                    nc.vector.tensor_mul(hts[d][:, :, col:col + BC],
                                         cells[d][:, 4:6, :],
                                         cells[d][:, 10:12, :])
            zwps.release()

            # ---- P3 + P4 interleaved: emission blocks outside-in, CRF
            # forward/backward chains chasing them.
            mmps = tc.alloc_tile_pool(name="mmps", bufs=2, space="PSUM")
            smps = tc.alloc_tile_pool(name="smps", bufs=2, space="PSUM")

            bstart = wpool.tile([T, 1], F32, tag="bstart")
            nc.vector.tensor_add(bstart[:], tab_sb[:, 78:79], tab_sb[:, 76:77])
            nc.sync.dma_start(em_sb[T:TA, :], padrow[:])
            em_accs = []

            def emit_emblock(tb):
                blk = slice(tb * 512, (tb + 1) * 512)
                ps = mmps.tile([T, 512], F32, tag="mm")
                for k in range(2):
                    nc.tensor.matmul(ps[:], wout_sb[:, k, :],
                                     hts[0][:, k, blk],
                                     start=(k == 0), stop=False)
                for k in range(2):
                    nc.tensor.matmul(ps[:], wout_sb[:, 2 + k, :],
                                     hts[1][:, k, blk],
                                     start=False, stop=(k == 1))
                acc = wpool.tile([T, 1], F32, tag="emacc" + str(tb), bufs=1,
                                 name=f"emacc{tb}")
                scr = wpool.tile([T, 512], F32, tag="ttrscr")
                nc.vector.tensor_mul(scr[:], ps[:], ohm_sb[:, blk])
                nc.vector.tensor_reduce(acc[:], scr[:],
                                        axis=mybir.AxisListType.X, op=ALU.add)
                em_accs.append(acc)
                # exp(em + b_out) straight from PSUM (first 32 cols add start)
                if tb == 0:
                    nc.scalar.activation(em_sb[0:T, 0:BC], ps[:, 0:BC],
                                         AF.Exp, bias=bstart[:])
                    nc.scalar.activation(em_sb[0:T, BC:512], ps[:, BC:512],
                                         AF.Exp, bias=tab_sb[:, 78:79])
                else:
                    nc.scalar.activation(em_sb[0:T, blk], ps[:],
                                         AF.Exp, bias=tab_sb[:, 78:79])
                nc.gpsimd.tensor_mul(em_sb[0:T, blk], em_sb[0:T, blk],
                                     vm_sb[:, blk])

            # CRF chains: fwd alpha t=1..64, bwd u_tau=126..65 then beta.
            af = em_sb[0:TA, 0:BC]
            ab = None
            bps = None

            def crf_fwd(tf):
                nonlocal af
                aps = smps.tile([TA, BC], F32, tag="smf")
                nc.tensor.matmul(aps[:, 0:BC], mp_sb[:], af,
                                 start=True, stop=True)
                an = spool.tile([TA, BC], BF16, tag="af")
                nc.vector.tensor_mul(an[:], aps[:, 0:BC],
                                     em_sb[0:TA, tf * BC:(tf + 1) * BC])
                af = an[:]

            def crf_bwd(i):
                nonlocal ab, bps
                if i == 0:
                    vb = spool.tile([TA, BC], BF16, tag="ab")
                    nc.vector.tensor_scalar_mul(
                        vb[:], em_sb[0:TA, (S - 1) * BC:S * BC],
                        eend_sb[:, 0:1])
                    ab = vb[:]
                    return
                aps = smps.tile([TA, BC], F32, tag="smb")
                nc.tensor.matmul(aps[:, 0:BC], mpT_sb[:], ab,
                                 start=True, stop=True)
                tau = 127 - i
                if tau >= 65:
                    an = spool.tile([TA, BC], BF16, tag="ab")
                    nc.vector.tensor_mul(an[:], aps[:, 0:BC],
                                         em_sb[0:TA, tau * BC:(tau + 1) * BC])
                    ab = an[:]
                else:                      # tau == 64: beta stays in PSUM
                    bps = aps

            emit_emblock(0)
            emit_emblock(7)
            crf_bwd(0)
            for i in range(15):            # fwd t=1..15, bwd tau=126..113
                crf_fwd(1 + i)
                crf_bwd(1 + i)
            for grp in range(3):           # blocks (1,6),(2,5),(3,4)
                emit_emblock(1 + grp)
                emit_emblock(6 - grp)
                for i in range(15 + grp * 16, 31 + grp * 16):
                    crf_fwd(1 + i)
                    crf_bwd(1 + i)
            for i in range(63, 64):
                crf_fwd(1 + i)             # t=64
            crf_bwd(63)                    # beta = M u_65 (psum)

            pb = wpool.tile([TA, BC], BF16, tag="pb")
            nc.vector.tensor_mul(pb[:], bps[:, 0:BC], af)
            sps = smps.tile([1, BC], F32, tag="smz")
            nc.tensor.matmul(sps[:, 0:BC], onesta[:], pb[:],
                             start=True, stop=True)
            logs = wpool.tile([1, BC], F32, tag="logs")
            nc.scalar.activation(logs[:], sps[:, 0:BC], AF.Ln)
            logsum = wpool.tile([1, 1], F32, tag="logsum")
            nc.vector.tensor_reduce(logsum[:], logs[:],
                                    axis=mybir.AxisListType.X, op=ALU.add)

            # gold score: table part
            gacc = wpool.tile([T, 1], F32, tag="gacc")
            scr2 = wpool.tile([T, 79], F32, tag="scr2")
            nc.vector.tensor_mul(scr2[:], gcnt_sb[:], tab_sb[:, 0:79])
            nc.vector.tensor_reduce(gacc[:], scr2[:],
                                    axis=mybir.AxisListType.X, op=ALU.add)
            tot = wpool.tile([T, 1], F32, tag="tot")
            nc.vector.tensor_add(tot[:], gacc[:], em_accs[0][:])
            for acc in em_accs[1:]:
                nc.vector.tensor_add(tot[:], tot[:], acc[:])
            ones = cpool.tile([T, 1], F32)
            nc.vector.memset(ones[:], 1.0)
            scps = smps.tile([1, 1], F32, tag="smz")
            nc.tensor.matmul(scps[:, 0:1], tot[:], ones[:],
                             start=True, stop=True)

            res = wpool.tile([1, 2], F32, tag="res")
            nc.vector.tensor_copy(res[:, 0:1], logsum[:])
            nc.vector.tensor_copy(res[:, 1:2], scps[:, 0:1])
            nc.sync.dma_start(out_d[:], res[:])
            smps.release()
            mmps.release()

    return nc


# ---------------------------------------------------------------- host side
def _gate_perm():
    """PyTorch gate order i,f,g,o -> reordered i,f,o,g (rows of W/b)."""
    return np.concatenate([
        np.arange(0, HD),            # i
        np.arange(HD, 2 * HD),       # f
        np.arange(3 * HD, 4 * HD),   # o
        np.arange(2 * HD, 3 * HD),   # g
    ])


def _pack_w_t(w, perm):
    """w: [G4, K] -> [128, K//128, 8, 128] bf16 with
    [p, k, c, m] = w[perm[c*128+m], k*128+p] (lhsT chunks).
    g-gate rows are doubled: the kernel computes tanh(g) as 2*sig(2g)-1."""
    wp = np.asarray(w)[perm, :].copy()
    wp[3 * HD:4 * HD, :] *= 2.0
    K = wp.shape[1]
    arr = wp.reshape(NC, 128, K // 128, 128)       # [c, m, k, p]
    return np.ascontiguousarray(arr.transpose(3, 2, 0, 1)).astype(
        ml_dtypes.bfloat16)


def prep_inputs(inputs):
    """Build per-core input maps + host constants."""
    ids = np.asarray(inputs["input_ids"])
    tags = np.asarray(inputs["tag_ids"])
    lengths = np.asarray(inputs["lengths"])
    perm = _gate_perm()

    embed_bf = np.asarray(inputs["embed_table"]).astype(ml_dtypes.bfloat16)

    def gather_xt(flat_ids):
        g = embed_bf[flat_ids]                       # [NTOK, E] bf16
        return np.ascontiguousarray(
            g.reshape(NTOK, 4, 128).transpose(2, 1, 0))

    wih_pack = np.stack([_pack_w_t(inputs["W_ih_f"], perm),
                         _pack_w_t(inputs["W_ih_b"], perm)], axis=1)
    whh_pack = np.stack([_pack_w_t(inputs["W_hh_f"], perm),
                         _pack_w_t(inputs["W_hh_b"], perm)], axis=1)
    wo = np.asarray(inputs["W_out"])          # [T, H]
    wout_pack = np.empty((128, 4, T), dtype=ml_dtypes.bfloat16)
    for k in range(4):
        wout_pack[:, k, :] = wo[:, k * 128:(k + 1) * 128].T.astype(
            ml_dtypes.bfloat16)
    bias_f = (np.asarray(inputs["b_ih_f"]) + np.asarray(inputs["b_hh_f"]))[perm]
    bias_b = (np.asarray(inputs["b_ih_b"]) + np.asarray(inputs["b_hh_b"]))[perm]
    bias_f[3 * HD:4 * HD] *= 2.0     # doubled g rows (tanh via 2*sig(2g)-1)
    bias_b[3 * HD:4 * HD] *= 2.0
    brow = np.stack([bias_f.reshape(NC, 128),
                     bias_b.reshape(NC, 128)])[None].astype(ml_dtypes.bfloat16)

    trans = np.asarray(inputs["trans"]).astype(np.float64)
    kappa = float(np.log(np.exp(trans).sum(axis=0).mean()))
    tables = np.zeros((T, 80), dtype=np.float32)
    tables[:, 0:T] = trans.astype(np.float32)
    tables[:, 76] = np.asarray(inputs["start_trans"])
    tables[:, 77] = np.asarray(inputs["end_trans"])
    tables[:, 78] = np.asarray(inputs["b_out"])
    tables[:, 79] = -kappa

    end_t = np.asarray(inputs["end_trans"]).astype(np.float64)
    mp = np.zeros((TA, TA), dtype=np.float64)
    mp[0:T, 0:T] = np.exp(trans - kappa)
    mp[0:T, T] = np.exp(end_t - kappa)
    mp[T, T] = 1.0
    mpT = mp.T.copy()
    eend = np.zeros((TA, 1), dtype=np.float32)
    eend[0:T, 0] = np.exp(end_t)
    eend[T, 0] = 1.0

    h0 = np.asarray(inputs["h0"])             # [2, B, HD]
    c0 = np.asarray(inputs["c0"])

    in_maps = []
    k_len_total = 0
    for ccore in range(N_CORES):
        bs = slice(ccore * BC, (ccore + 1) * BC)
        ids_c = ids[bs]
        tags_c = tags[bs]
        len_c = lengths[bs].astype(np.int64)
        k_len_total += int(np.minimum(len_c, S - 1).sum())

        idx_f = ids_c.T.reshape(-1)                    # token (s, b) order
        xt = gather_xt(idx_f)

        svec = np.arange(S)[None, :]
        valid = (svec < len_c[:, None]).T.reshape(-1)  # [(s, b)]
        ohm = np.zeros((T, NTOK), dtype=ml_dtypes.bfloat16)
        tt = tags_c.T.reshape(-1)
        pos = np.arange(NTOK)
        ohm[tt[valid], pos[valid]] = 1
        vm = np.broadcast_to(valid.astype(ml_dtypes.bfloat16),
                             (T, NTOK)).copy()
        padr = (~valid).astype(ml_dtypes.bfloat16)[None, :]

        Cm = np.zeros((T, T), dtype=np.float32)
        h0v = np.zeros(T, dtype=np.float32)
        hLv = np.zeros(T, dtype=np.float32)
        for b in range(BC):
            L = int(len_c[b])
            tg = tags_c[b, :L]
            np.add.at(Cm, (tg[:-1], tg[1:]), 1)
            h0v[tg[0]] += 1
            hLv[tg[-1]] += 1
        nv = ohm.astype(np.float32).sum(axis=1)
        gcnt = np.concatenate([Cm, h0v[:, None], hLv[:, None], nv[:, None]],
                              axis=1)

        h0t = np.stack(
            [h0[d][bs].reshape(BC, 2, 128).transpose(2, 1, 0)
             for d in range(2)], axis=1).astype(ml_dtypes.bfloat16)
        c0t = np.stack(
            [c0[d][bs].reshape(BC, 2, 128).transpose(2, 1, 0)
             for d in range(2)], axis=1).astype(np.float32)

        in_maps.append(dict(
            xt=xt, wih=wih_pack, whh=whh_pack, brow=brow,
            wout=wout_pack, h0t=h0t, c0t=c0t,
            tables=tables, gcnt=gcnt.astype(np.float32), ohm=ohm,
            vmask=vm, padrow=padr,
            mp=mp.astype(ml_dtypes.bfloat16),
            mpT=mpT.astype(ml_dtypes.bfloat16), eend=eend,
        ))

    return in_maps, dict(kappa=kappa, k_len_total=k_len_total)


def finalize(results, host):
    logz = sum(float(r["out"][0, 0]) for r in results)
    score = sum(float(r["out"][0, 1]) for r in results)
    logz += host["kappa"] * host["k_len_total"]
    return np.float32((logz - score) / B)


# ---------------------------------------------------------------- entry point
_COMPILED = {}


def kernel(**inputs):
    """Full-input BiLSTM-CRF loss on 8 NeuronCores (data parallel)."""
    from concourse.bass_utils import run_bass_kernel_spmd
    in_maps, host = prep_inputs(inputs)
    if "nc" not in _COMPILED:
        _COMPILED["nc"] = build_nc()
    nc = _COMPILED["nc"]
    res = run_bass_kernel_spmd(nc, in_maps, core_ids=list(range(N_CORES)))
    return np.asarray(finalize(res.results, host))
